# revision 8
# baseline (speedup 1.0000x reference)
"""Causal self-attention with anchor-relative rope (ferope), 8-core TRN2 Bass kernel.

Full-scale problem: B=2, T=2048, C=2048, H=16, D=128, M=32.

Sharding (tensor-parallel heads + data-parallel batch):
  - 8 cores = 2 batch groups x 4 cores. Core (b, g) handles batch b, heads 4g..4g+3.
  - qkv projection: each core computes q/k/v only for its heads (w_attn column shard),
    from x[b] transposed (host prep) so the contraction dim c sits on partitions.
  - attention computed with scores transposed: s_T[ki,qi], so both attention
    matmuls contract along partitions with no on-device transposes.
  - y_T head slices ([128, T] per head, c-major) are AllGathered within each
    4-core batch group -> y_all [C, T].
  - output projection is column-sharded: each core computes out[b][:, g*512:(g+1)*512].

All matmuls run as float32r (1 cycle/row at N>=512) except the qkv projection,
whose inputs (xT and w_attn shards) are cast to bf16 on device to fit SBUF.
"""

import math

import numpy as np

import concourse.bass as bass
import concourse.mybir as mybir
import concourse.tile as tile
from concourse import bacc
from concourse.bass_utils import run_bass_kernel_spmd

F32 = mybir.dt.float32
F32R = mybir.dt.float32r
BF16 = mybir.dt.bfloat16

# full-scale dims (hardcoded per harness contract)
B, T, C, H, DH, M = 2, 2048, 2048, 16, 128, 32
N_CORES = 8
GROUPS = 2                     # batch groups
CPG = N_CORES // GROUPS        # cores per group = 4
HPC = H // CPG                 # heads per core = 4
C_LOC = HPC * DH               # 512: per-core head channels
PANEL = 512                    # qi panel width (one psum bank)
KB = 128                       # ki block (partition dim)


def r(ap):
    """View a float32 AP as float32r for full-rate matmul."""
    return ap.bitcast(F32R)


def build_program(T=T, C=C, HPC=HPC, DH=DH, M=M, n_cores=N_CORES, groups=GROUPS):
    """Build the SPMD Bass program (same program on all cores; data differs)."""
    cpg = n_cores // groups
    c_loc = HPC * DH
    n_cb = C // KB            # contraction blocks for qkv/proj
    n_panels = T // PANEL
    n_tb = T // KB
    kb_per_panel = PANEL // KB  # 4
    inv_sqrt_d = 1.0 / math.sqrt(DH)

    nc = bacc.Bacc("TRN2", target_bir_lowering=False, debug=False,
                   num_devices=n_cores)

    xT_d = nc.dram_tensor("xT", [C, T], F32, kind="ExternalInput").ap()
    wqk_d = nc.dram_tensor("wqk", [C, 2 * c_loc], F32, kind="ExternalInput").ap()
    wv_d = nc.dram_tensor("wv", [C, c_loc], F32, kind="ExternalInput").ap()
    wo_d = nc.dram_tensor("wo", [C, c_loc], F32R, kind="ExternalInput").ap()
    freqs_d = nc.dram_tensor("freqs", [M], F32, kind="ExternalInput").ap()
    delta_d = nc.dram_tensor("delta", [T], F32, kind="ExternalInput").ap()
    out_d = nc.dram_tensor("out", [T, c_loc], F32, kind="ExternalOutput").ap()

    replica_groups = [list(range(g * cpg, (g + 1) * cpg)) for g in range(groups)]

    with tile.TileContext(nc) as tc:
        with (
            tc.tile_pool(name="dram", bufs=1, space="DRAM") as dram,
            tc.tile_pool(name="const", bufs=1) as const,
            tc.tile_pool(name="qkv", bufs=1) as qkv,
            tc.tile_pool(name="work", bufs=1) as work,
        ):
            y_part = dram.tile([c_loc, T], F32R)
            y_all = dram.tile([cpg * c_loc, T], F32R)

            # ---- constants: trig tables, causal masks, ones ----
            ones_col = const.tile([KB, 1], BF16)
            nc.vector.memset(ones_col[:], 1.0)
            ones_row = const.tile([1, KB], F32)
            nc.vector.memset(ones_row[:], 1.0)

            sinN = const.tile([2 * M, T], F32)
            cos64 = const.tile([2 * M, T], F32)
            masks = [const.tile([KB, PANEL], F32, name=f"maskf{p}")
                     for p in range(kb_per_panel)]
            with tc.tile_pool(name="setup", bufs=1) as setup:
                # fr64 = [-freqs; freqs] as per-partition scalars
                fr64 = setup.tile([2 * M, 1], F32)
                nc.sync.dma_start(out=fr64[0:M, :],
                                  in_=freqs_d.rearrange("m -> m ()"))
                nc.sync.dma_start(out=fr64[M:2 * M, :],
                                  in_=freqs_d.rearrange("m -> m ()"))
                nc.vector.tensor_scalar_mul(fr64[0:M, :], fr64[0:M, :], -1.0)

                # delta broadcast across 2M partitions
                delta_row = setup.tile([1, T], F32)
                nc.sync.dma_start(out=delta_row[:],
                                  in_=delta_d.rearrange("t -> () t"))
                delta_rep = setup.tile([2 * M, T], F32)
                nc.gpsimd.partition_broadcast(delta_rep[:], delta_row[:],
                                              channels=2 * M)

                # ang = delta * (+-freqs); sinN = [-sin; sin], cos = [cos; cos]
                ang = setup.tile([2 * M, T], F32)
                nc.vector.tensor_scalar_mul(ang[:], delta_rep[:], fr64[:])
                nc.scalar.activation(sinN[:], ang[:],
                                     mybir.ActivationFunctionType.Sin)
                pi2 = setup.tile([2 * M, 1], F32)
                nc.vector.memset(pi2[:], math.pi / 2)
                nc.scalar.activation(cos64[:], ang[:],
                                     mybir.ActivationFunctionType.Sin,
                                     bias=pi2[:])

                # causal masks for diagonal tiles: mask_p = (qi >= ki + 128*p)
                for p in range(kb_per_panel):
                    mi = setup.tile([KB, PANEL], F32, tag="maski", bufs=2,
                                    name=f"maski{p}")
                    nc.gpsimd.iota(mi[:], pattern=[[1, PANEL]], base=-KB * p,
                                   channel_multiplier=-1,
                                   allow_small_or_imprecise_dtypes=True)
                    nc.vector.tensor_scalar(masks[p][:], mi[:], 0.0, None,
                                            mybir.AluOpType.is_ge)

            # ---- qkv projection for all local heads, single pass over xT ----
            # q/k stored per head as [d, t] bf16; v natural [t, d] bf16.
            q_sb = [qkv.tile([DH, T], BF16, name=f"q{h}") for h in range(HPC)]
            k_sb = [qkv.tile([DH, T], BF16, name=f"k{h}") for h in range(HPC)]
            v_all = qkv.tile([KB, n_tb, c_loc], BF16)
            with tc.tile_pool(name="wload", bufs=1) as wload:
                wqkb, wvb = [], []
                for i in range(n_cb):
                    ws = wload.tile([KB, 2 * c_loc], F32, tag="wstage", bufs=2,
                                    name=f"wqs{i}")
                    nc.sync.dma_start(out=ws[:],
                                      in_=wqk_d[i * KB:(i + 1) * KB, :])
                    wb = wload.tile([KB, 2 * c_loc], BF16, tag="wqkbf",
                                    bufs=n_cb, name=f"wqkb{i}")
                    nc.vector.tensor_copy(wb[:], ws[:])
                    wqkb.append(wb)
                for i in range(n_cb):
                    ws = wload.tile([KB, c_loc], F32, tag="wvstage", bufs=2,
                                    name=f"wvs{i}")
                    nc.sync.dma_start(out=ws[:],
                                      in_=wv_d[i * KB:(i + 1) * KB, :])
                    wv_bf = wload.tile([KB, c_loc], BF16, tag="wvbf",
                                       bufs=n_cb, name=f"wvb{i}")
                    nc.vector.tensor_copy(wv_bf[:], ws[:])
                    wvb.append(wv_bf)
                with tc.tile_pool(name="psq", bufs=1, space="PSUM") as psq:
                    for tp in range(n_panels):
                        tps = tp * PANEL
                        xbf = []
                        for kb in range(n_cb):
                            xs = wload.tile([KB, PANEL], F32, tag="xstage",
                                            bufs=3, name=f"xs{tp}_{kb}")
                            nc.sync.dma_start(
                                out=xs[:],
                                in_=xT_d[kb * KB:(kb + 1) * KB,
                                         tps:tps + PANEL])
                            xb = wload.tile([KB, PANEL], BF16, tag="xbf",
                                            bufs=n_cb, name=f"xb{tp}_{kb}")
                            nc.vector.tensor_copy(xb[:], xs[:])
                            xbf.append(xb)
                        # q/k column blocks: cb<HPC -> q head cb; else k head
                        for cb in range(2 * HPC):
                            pqk = psq.tile([DH, PANEL], F32, tag="qk", bufs=3)
                            for kb in range(n_cb):
                                nc.tensor.matmul(
                                    pqk[:],
                                    wqkb[kb][:, cb * DH:(cb + 1) * DH],
                                    xbf[kb][:],
                                    start=(kb == 0), stop=(kb == n_cb - 1))
                            dst = q_sb[cb] if cb < HPC else k_sb[cb - HPC]
                            nc.scalar.copy(dst[:, tps:tps + PANEL], pqk[:])
                        # v blocks for the 128-rows inside this panel
                        for tbl in range(kb_per_panel):
                            tb = tp * kb_per_panel + tbl
                            pv = psq.tile([KB, c_loc], F32, tag="v", bufs=3)
                            for kb in range(n_cb):
                                nc.tensor.matmul(
                                    pv[:],
                                    xbf[kb][:, tbl * KB:(tbl + 1) * KB],
                                    wvb[kb][:],
                                    start=(kb == 0), stop=(kb == n_cb - 1))
                            nc.scalar.copy(v_all[:, tb, :], pv[:])

            # ---- rope on rows 0:2M of each q/k head ----
            for u in [t for pair in zip(q_sb, k_sb) for t in pair]:
                sw = work.tile([2 * M, T], BF16, tag="ropesw", bufs=2)
                nc.sync.dma_start(out=sw[0:M, :], in_=u[M:2 * M, :])
                nc.sync.dma_start(out=sw[M:2 * M, :], in_=u[0:M, :])
                nc.vector.tensor_mul(sw[:], sw[:], sinN[:])
                nc.vector.tensor_mul(u[0:2 * M, :], u[0:2 * M, :], cos64[:])
                nc.vector.tensor_add(u[0:2 * M, :], u[0:2 * M, :], sw[:])

            # ---- causal attention per head, qi panels of 512 ----
            with tc.tile_pool(name="psa", bufs=1, space="PSUM") as psa:
                for h in range(HPC):
                    qh, kh = q_sb[h], k_sb[h]
                    for J in range(n_panels):
                        nkb = (J + 1) * kb_per_panel
                        py = psa.tile([DH, PANEL], F32, tag="y", bufs=2)
                        pr = psa.tile([1, PANEL], F32, tag="r", bufs=2)
                        qs = J * PANEL
                        for b in range(nkb):
                            ps = psa.tile([KB, PANEL], F32, tag="s", bufs=2)
                            nc.tensor.matmul(
                                ps[:],
                                kh[:, b * KB:(b + 1) * KB],
                                qh[:, qs:qs + PANEL],
                                start=True, stop=True)
                            et = work.tile([KB, PANEL], BF16, tag="exp",
                                           bufs=3)
                            nc.scalar.activation(
                                et[:], ps[:],
                                mybir.ActivationFunctionType.Exp,
                                scale=inv_sqrt_d)
                            p = b - kb_per_panel * J
                            if p >= 0:
                                nc.vector.tensor_mul(et[:], et[:], masks[p][:])
                            nc.tensor.matmul(
                                py[:],
                                v_all[:, b, h * DH:(h + 1) * DH],
                                et[:],
                                start=(b == 0), stop=(b == nkb - 1))
                            nc.tensor.matmul(
                                pr[:], ones_col[:], et[:],
                                start=(b == 0), stop=(b == nkb - 1))
                        # normalize: y / rowsum (broadcast over partitions)
                        rinv = work.tile([1, PANEL], F32, tag="rinv", bufs=2)
                        nc.vector.reciprocal(rinv[:], pr[:])
                        prep = psa.tile([KB, PANEL], F32, tag="rep", bufs=2)
                        nc.tensor.matmul(prep[:], ones_row[:], rinv[:],
                                         start=True, stop=True)
                        rep = work.tile([KB, PANEL], F32, tag="rep_sb", bufs=2)
                        nc.scalar.copy(rep[:], prep[:])
                        ysb = work.tile([DH, PANEL], F32, tag="ysb", bufs=3)
                        nc.vector.tensor_mul(ysb[:], py[:], rep[:])
                        nc.sync.dma_start(
                            out=y_part[h * DH:(h + 1) * DH, qs:qs + PANEL],
                            in_=r(ysb[:]))

            # ---- AllGather y within batch group ----
            nc.gpsimd.collective_compute(
                "AllGather",
                mybir.AluOpType.bypass,
                replica_groups=replica_groups,
                ins=[y_part.opt()],
                outs=[y_all.opt()],
            )

            # ---- out[:, local cols] = y_all.T @ wo ----
            with (
                tc.tile_pool(name="proj", bufs=1) as proj,
                tc.tile_pool(name="pso", bufs=1, space="PSUM") as pso,
            ):
                wob = []
                for i in range(n_cb):
                    wo_sb = proj.tile([KB, c_loc], F32R, tag="wo", bufs=n_cb,
                                      name=f"wo{i}")
                    nc.sync.dma_start(out=wo_sb[:],
                                      in_=wo_d[i * KB:(i + 1) * KB, :])
                    wob.append(wo_sb)
                for tb in range(n_tb):
                    yt = proj.tile([KB, n_cb, KB], F32R, tag="yt", bufs=2)
                    for cb in range(n_cb):
                        nc.sync.dma_start(
                            out=yt[:, cb, :],
                            in_=y_all[cb * KB:(cb + 1) * KB,
                                      tb * KB:(tb + 1) * KB])
                    po = pso.tile([KB, c_loc], F32, tag="o", bufs=4)
                    for cb in range(n_cb):
                        nc.tensor.matmul(po[:], yt[:, cb, :],
                                         wob[cb][:],
                                         start=(cb == 0), stop=(cb == n_cb - 1))
                    osb = proj.tile([KB, c_loc], F32, tag="osb", bufs=3)
                    nc.scalar.copy(osb[:], po[:])
                    nc.sync.dma_start(out=out_d[tb * KB:(tb + 1) * KB, :],
                                      in_=osb[:])

    nc.compile()
    return nc


def make_in_maps(x, w_attn, w_proj, freqs, delta, n_cores=N_CORES,
                 groups=GROUPS, dh=DH):
    """Host-side sharding: slice/transpose full inputs into per-core maps."""
    x = np.asarray(x, dtype=np.float32)
    w_attn = np.asarray(w_attn, dtype=np.float32)
    w_proj = np.asarray(w_proj, dtype=np.float32)
    freqs = np.asarray(freqs, dtype=np.float32)
    delta = np.asarray(delta, dtype=np.float32)
    b_, t_, c_ = x.shape
    cpg = n_cores // groups
    h_ = w_attn.shape[1] // (3 * dh)
    hpc = h_ // cpg
    c_loc = hpc * dh
    in_maps = []
    for core in range(n_cores):
        g, pos = divmod(core, cpg)
        heads = range(pos * hpc, (pos + 1) * hpc)
        xT = np.ascontiguousarray(x[g].T)
        wqk = np.concatenate(
            [w_attn[:, h * dh:(h + 1) * dh] for h in heads]
            + [w_attn[:, c_ + h * dh:c_ + (h + 1) * dh] for h in heads], axis=1)
        wv = np.concatenate(
            [w_attn[:, 2 * c_ + h * dh:2 * c_ + (h + 1) * dh] for h in heads],
            axis=1)
        wo = np.ascontiguousarray(w_proj[:, pos * c_loc:(pos + 1) * c_loc])
        in_maps.append({
            "xT": xT,
            "wqk": np.ascontiguousarray(wqk),
            "wv": np.ascontiguousarray(wv),
            "wo": wo,
            "freqs": freqs,
            "delta": delta,
        })
    return in_maps


def assemble_output(results, n_cores=N_CORES, groups=GROUPS):
    cpg = n_cores // groups
    outs = []
    for g in range(groups):
        cols = [results[g * cpg + pos]["out"] for pos in range(cpg)]
        outs.append(np.concatenate(cols, axis=1))
    return np.stack(outs, axis=0).astype(np.float32)


_NC_CACHE = {}


def _get_program():
    if "nc" not in _NC_CACHE:
        _NC_CACHE["nc"] = build_program()
    return _NC_CACHE["nc"]


def kernel(x, w_attn, w_proj, freqs, delta):
    nc = _get_program()
    in_maps = make_in_maps(x, w_attn, w_proj, freqs, delta)
    res = run_bass_kernel_spmd(nc, in_maps, list(range(N_CORES)))
    return assemble_output(res.results)


# revision 13
# speedup vs baseline: 1.3758x; 1.3758x over previous
"""Causal self-attention with anchor-relative rope (ferope), 8-core TRN2 Bass kernel.

Full-scale problem: B=2, T=2048, C=2048, H=16, D=128, M=32.

Sharding (tensor-parallel heads + data-parallel batch):
  - 8 cores = 2 batch groups x 4 cores. Core (b, g) handles batch b, heads 4g..4g+3.
  - qkv projection: each core computes q/k/v only for its heads (w_attn column shard),
    from x[b] transposed (host prep) so the contraction dim c sits on partitions.
  - attention computed with scores transposed: s_T[ki,qi], so both attention
    matmuls contract along partitions with no on-device transposes.
  - y_T head slices ([128, T] per head, c-major) are AllGathered within each
    4-core batch group -> y_all [C, T].
  - output projection is column-sharded: each core computes out[b][:, g*512:(g+1)*512].

All matmuls run as float32r (1 cycle/row at N>=512) except the qkv projection,
whose inputs (xT and w_attn shards) are cast to bf16 on device to fit SBUF.
"""

import math

import numpy as np

import concourse.bass as bass
import concourse.mybir as mybir
import concourse.tile as tile
from concourse import bacc
from concourse.bass_utils import run_bass_kernel_spmd

F32 = mybir.dt.float32
F32R = mybir.dt.float32r
BF16 = mybir.dt.bfloat16

# full-scale dims (hardcoded per harness contract)
B, T, C, H, DH, M = 2, 2048, 2048, 16, 128, 32
N_CORES = 8
GROUPS = 2                     # batch groups
CPG = N_CORES // GROUPS        # cores per group = 4
HPC = H // CPG                 # heads per core = 4
C_LOC = HPC * DH               # 512: per-core head channels
PANEL = 512                    # qi panel width (one psum bank)
KB = 128                       # ki block (partition dim)


def r(ap):
    """View a float32 AP as float32r for full-rate matmul."""
    return ap.bitcast(F32R)


def build_program(T=T, C=C, HPC=HPC, DH=DH, M=M, n_cores=N_CORES, groups=GROUPS):
    """Build the SPMD Bass program (same program on all cores; data differs)."""
    cpg = n_cores // groups
    c_loc = HPC * DH
    n_cb = C // KB            # contraction blocks for qkv/proj
    n_panels = T // PANEL
    n_tb = T // KB
    kb_per_panel = PANEL // KB  # 4
    inv_sqrt_d = 1.0 / math.sqrt(DH)

    nc = bacc.Bacc("TRN2", target_bir_lowering=False, debug=False,
                   num_devices=n_cores)

    xT_d = nc.dram_tensor("xT", [C, T], F32, kind="ExternalInput").ap()
    wqk_d = nc.dram_tensor("wqk", [C, 2 * c_loc], F32, kind="ExternalInput").ap()
    wv_d = nc.dram_tensor("wv", [C, c_loc], F32, kind="ExternalInput").ap()
    wo_d = nc.dram_tensor("wo", [C, c_loc], F32R, kind="ExternalInput").ap()
    freqs_d = nc.dram_tensor("freqs", [M], F32, kind="ExternalInput").ap()
    delta_d = nc.dram_tensor("delta", [T], F32, kind="ExternalInput").ap()
    out_d = nc.dram_tensor("out", [T, c_loc], F32, kind="ExternalOutput").ap()

    replica_groups = [list(range(g * cpg, (g + 1) * cpg)) for g in range(groups)]

    with tile.TileContext(nc) as tc:
        with (
            tc.tile_pool(name="dram", bufs=1, space="DRAM") as dram,
            tc.tile_pool(name="const", bufs=1) as const,
            tc.tile_pool(name="qkv", bufs=1) as qkv,
            tc.tile_pool(name="work", bufs=1) as work,
        ):
            y_part = dram.tile([c_loc, T], F32R)
            y_all = dram.tile([cpg * c_loc, T], F32R)

            # ---- constants: trig tables, causal masks, ones ----
            ones128 = const.tile([KB, KB], BF16)
            nc.vector.memset(ones128[:], 1.0)

            sinN = const.tile([2 * M, T], F32)
            cos64 = const.tile([2 * M, T], F32)
            masks = [const.tile([KB, PANEL], F32, name=f"maskf{p}")
                     for p in range(kb_per_panel)]
            with tc.tile_pool(name="setup", bufs=1) as setup:
                # fr64 = [-freqs; freqs] as per-partition scalars
                fr64 = setup.tile([2 * M, 1], F32)
                nc.sync.dma_start(out=fr64[0:M, :],
                                  in_=freqs_d.rearrange("m -> m ()"))
                nc.sync.dma_start(out=fr64[M:2 * M, :],
                                  in_=freqs_d.rearrange("m -> m ()"))
                nc.vector.tensor_scalar_mul(fr64[0:M, :], fr64[0:M, :], -1.0)

                # delta broadcast across 2M partitions
                delta_row = setup.tile([1, T], F32)
                nc.sync.dma_start(out=delta_row[:],
                                  in_=delta_d.rearrange("t -> () t"))
                delta_rep = setup.tile([2 * M, T], F32)
                nc.gpsimd.partition_broadcast(delta_rep[:], delta_row[:],
                                              channels=2 * M)

                # ang = delta * (+-freqs); sinN = [-sin; sin], cos = [cos; cos]
                ang = setup.tile([2 * M, T], F32)
                nc.vector.tensor_scalar_mul(ang[:], delta_rep[:], fr64[:])
                nc.scalar.activation(sinN[:], ang[:],
                                     mybir.ActivationFunctionType.Sin)
                pi2 = setup.tile([2 * M, 1], F32)
                nc.vector.memset(pi2[:], math.pi / 2)
                nc.scalar.activation(cos64[:], ang[:],
                                     mybir.ActivationFunctionType.Sin,
                                     bias=pi2[:])

                # causal masks for diagonal tiles: mask_p = (qi >= ki + 128*p)
                for p in range(kb_per_panel):
                    mi = setup.tile([KB, PANEL], F32, tag="maski", bufs=2,
                                    name=f"maski{p}")
                    nc.gpsimd.iota(mi[:], pattern=[[1, PANEL]], base=-KB * p,
                                   channel_multiplier=-1,
                                   allow_small_or_imprecise_dtypes=True)
                    nc.vector.tensor_scalar(masks[p][:], mi[:], 0.0, None,
                                            mybir.AluOpType.is_ge)

            # ---- qkv projection for all local heads, single pass over xT ----
            # q/k stored per head as [d, t] bf16; v natural [t, d] bf16.
            q_sb = [qkv.tile([DH, T], BF16, name=f"q{h}") for h in range(HPC)]
            k_sb = [qkv.tile([DH, T], BF16, name=f"k{h}") for h in range(HPC)]
            v_all = qkv.tile([KB, n_tb, c_loc], BF16)
            with tc.tile_pool(name="wload", bufs=1) as wload:
                wqkb, wvb = [], []
                for i in range(n_cb):
                    ws = wload.tile([KB, c_loc], F32, tag="wvstage", bufs=2,
                                    name=f"wvs{i}")
                    nc.sync.dma_start(out=ws[:],
                                      in_=wv_d[i * KB:(i + 1) * KB, :])
                    wv_bf = wload.tile([KB, c_loc], BF16, tag="wvbf",
                                       bufs=n_cb, name=f"wvb{i}")
                    nc.vector.tensor_copy(wv_bf[:], ws[:])
                    wvb.append(wv_bf)
                for i in range(n_cb):
                    ws = wload.tile([KB, 2 * c_loc], F32, tag="wstage", bufs=2,
                                    name=f"wqs{i}")
                    nc.sync.dma_start(out=ws[:],
                                      in_=wqk_d[i * KB:(i + 1) * KB, :])
                    wb = wload.tile([KB, 2 * c_loc], BF16, tag="wqkbf",
                                    bufs=n_cb, name=f"wqkb{i}")
                    nc.vector.tensor_copy(wb[:], ws[:])
                    wqkb.append(wb)
                with tc.tile_pool(name="psq", bufs=1, space="PSUM") as psq:
                    for tp in range(n_panels):
                        tps = tp * PANEL
                        xbf = []
                        for kb in range(n_cb):
                            xs = wload.tile([KB, PANEL], F32, tag="xstage",
                                            bufs=3, name=f"xs{tp}_{kb}")
                            nc.sync.dma_start(
                                out=xs[:],
                                in_=xT_d[kb * KB:(kb + 1) * KB,
                                         tps:tps + PANEL])
                            xb = wload.tile([KB, PANEL], BF16, tag="xbf",
                                            bufs=n_cb, name=f"xb{tp}_{kb}")
                            nc.vector.tensor_copy(xb[:], xs[:])
                            xbf.append(xb)
                        # v blocks for the 128-rows inside this panel
                        for tbl in range(kb_per_panel):
                            tb = tp * kb_per_panel + tbl
                            pv = psq.tile([KB, c_loc], F32, tag="v", bufs=3)
                            for kb in range(n_cb):
                                nc.tensor.matmul(
                                    pv[:],
                                    xbf[kb][:, tbl * KB:(tbl + 1) * KB],
                                    wvb[kb][:],
                                    start=(kb == 0), stop=(kb == n_cb - 1))
                            nc.scalar.copy(v_all[:, tb, :], pv[:])
                        # q/k column blocks: cb<HPC -> q head cb; else k head
                        for cb in range(2 * HPC):
                            pqk = psq.tile([DH, PANEL], F32, tag="qk", bufs=3)
                            for kb in range(n_cb):
                                nc.tensor.matmul(
                                    pqk[:],
                                    wqkb[kb][:, cb * DH:(cb + 1) * DH],
                                    xbf[kb][:],
                                    start=(kb == 0), stop=(kb == n_cb - 1))
                            dst = q_sb[cb] if cb < HPC else k_sb[cb - HPC]
                            nc.scalar.copy(dst[:, tps:tps + PANEL], pqk[:])

            # ---- rope on rows 0:2M of each q/k head ----
            for u in [t for pair in zip(q_sb, k_sb) for t in pair]:
                sw = work.tile([2 * M, T], BF16, tag="ropesw", bufs=2)
                nc.sync.dma_start(out=sw[0:M, :], in_=u[M:2 * M, :])
                nc.sync.dma_start(out=sw[M:2 * M, :], in_=u[0:M, :])
                nc.vector.tensor_mul(sw[:], sw[:], sinN[:])
                nc.vector.tensor_mul(u[0:2 * M, :], u[0:2 * M, :], cos64[:])
                nc.vector.tensor_add(u[0:2 * M, :], u[0:2 * M, :], sw[:])

            # ---- causal attention per head + per-head AllGather ----
            # y_all rows are head-major: (head, group, d) so each per-head
            # gather writes one contiguous [cpg*DH, T] block
            with tc.tile_pool(name="proj", bufs=1) as proj:
                # prefetch proj weights during attention
                wob = []
                for i in range(n_cb):
                    wo_sb = proj.tile([KB, c_loc], F32R, tag="wo", bufs=n_cb,
                                      name=f"wo{i}")
                    nc.sync.dma_start(out=wo_sb[:],
                                      in_=wo_d[i * KB:(i + 1) * KB, :])
                    wob.append(wo_sb)
                psa_cm = tc.tile_pool(name="psa", bufs=1, space="PSUM")
                psa = psa_cm.__enter__()
                for h in range(HPC):
                    qh, kh = q_sb[h], k_sb[h]
                    for J in range(n_panels):
                        nkb = (J + 1) * kb_per_panel
                        py = psa.tile([DH, PANEL], F32, tag="y", bufs=3)
                        pr = psa.tile([KB, PANEL], F32, tag="r", bufs=2)
                        qs = J * PANEL
                        for b in range(nkb):
                            ps = psa.tile([KB, PANEL], F32, tag="s", bufs=3)
                            nc.tensor.matmul(
                                ps[:],
                                kh[:, b * KB:(b + 1) * KB],
                                qh[:, qs:qs + PANEL],
                                start=True, stop=True)
                            et = work.tile([KB, PANEL], BF16, tag="exp",
                                           bufs=4)
                            nc.scalar.activation(
                                et[:], ps[:],
                                mybir.ActivationFunctionType.Exp,
                                scale=inv_sqrt_d)
                            p = b - kb_per_panel * J
                            if p >= 0:
                                nc.vector.tensor_mul(et[:], et[:], masks[p][:])
                            nc.tensor.matmul(
                                py[:],
                                v_all[:, b, h * DH:(h + 1) * DH],
                                et[:],
                                start=(b == 0), stop=(b == nkb - 1))
                            # rowsum, pre-replicated across partitions by
                            # using an all-ones [128,128] stationary operand
                            nc.tensor.matmul(
                                pr[:], ones128[:], et[:],
                                start=(b == 0), stop=(b == nkb - 1))
                        # normalize: y * (1/rowsum)
                        rep = work.tile([KB, PANEL], F32, tag="rep_sb", bufs=2)
                        nc.scalar.copy(rep[:], pr[:])
                        rinv = work.tile([KB, PANEL], F32, tag="rinv", bufs=2)
                        nc.vector.reciprocal_approx_fast(rinv[:], rep[:])
                        ysb = work.tile([DH, PANEL], F32, tag="ysb", bufs=3)
                        nc.vector.tensor_mul(ysb[:], py[:], rinv[:])
                        nc.sync.dma_start(
                            out=y_part[h * DH:(h + 1) * DH, qs:qs + PANEL],
                            in_=r(ysb[:]))
                    # gather this head's y slice across the batch group
                    nc.gpsimd.collective_compute(
                        "AllGather",
                        mybir.AluOpType.bypass,
                        replica_groups=replica_groups,
                        ins=[y_part[h * DH:(h + 1) * DH, :]],
                        outs=[y_all[h * cpg * DH:(h + 1) * cpg * DH, :]],
                    )

                psa_cm.__exit__(None, None, None)
                # ---- out[:, local cols] = y_all.T @ wo ----
                # y_all row (hh, g, p) -> c-block cb = g*HPC + hh
                y_all_tiled = y_all[:].rearrange(
                    "(hh g p) t -> p g hh t", hh=HPC, g=cpg)
                with tc.tile_pool(name="pso", bufs=1, space="PSUM") as pso:
                    for tb in range(n_tb):
                        yt = proj.tile([KB, cpg, HPC, KB], F32R, tag="yt",
                                       bufs=3)
                        for g in range(cpg):
                            nc.sync.dma_start(
                                out=yt[:, g],
                                in_=y_all_tiled[:, g, :,
                                                tb * KB:(tb + 1) * KB])
                        po = pso.tile([KB, c_loc], F32, tag="o", bufs=4)
                        for cb in range(n_cb):
                            nc.tensor.matmul(po[:], yt[:, cb // HPC, cb % HPC, :],
                                             wob[cb][:],
                                             start=(cb == 0),
                                             stop=(cb == n_cb - 1))
                        osb = proj.tile([KB, c_loc], F32, tag="osb", bufs=3)
                        nc.scalar.copy(osb[:], po[:])
                        nc.sync.dma_start(out=out_d[tb * KB:(tb + 1) * KB, :],
                                          in_=osb[:])

    nc.compile()
    return nc


def make_in_maps(x, w_attn, w_proj, freqs, delta, n_cores=N_CORES,
                 groups=GROUPS, dh=DH):
    """Host-side sharding: slice/transpose full inputs into per-core maps."""
    x = np.asarray(x, dtype=np.float32)
    w_attn = np.asarray(w_attn, dtype=np.float32)
    w_proj = np.asarray(w_proj, dtype=np.float32)
    freqs = np.asarray(freqs, dtype=np.float32)
    delta = np.asarray(delta, dtype=np.float32)
    b_, t_, c_ = x.shape
    cpg = n_cores // groups
    h_ = w_attn.shape[1] // (3 * dh)
    hpc = h_ // cpg
    c_loc = hpc * dh
    in_maps = []
    for core in range(n_cores):
        g, pos = divmod(core, cpg)
        heads = range(pos * hpc, (pos + 1) * hpc)
        xT = np.ascontiguousarray(x[g].T)
        wqk = np.concatenate(
            [w_attn[:, h * dh:(h + 1) * dh] for h in heads]
            + [w_attn[:, c_ + h * dh:c_ + (h + 1) * dh] for h in heads], axis=1)
        wv = np.concatenate(
            [w_attn[:, 2 * c_ + h * dh:2 * c_ + (h + 1) * dh] for h in heads],
            axis=1)
        wo = np.ascontiguousarray(w_proj[:, pos * c_loc:(pos + 1) * c_loc])
        in_maps.append({
            "xT": xT,
            "wqk": np.ascontiguousarray(wqk),
            "wv": np.ascontiguousarray(wv),
            "wo": wo,
            "freqs": freqs,
            "delta": delta,
        })
    return in_maps


def assemble_output(results, n_cores=N_CORES, groups=GROUPS):
    cpg = n_cores // groups
    outs = []
    for g in range(groups):
        cols = [results[g * cpg + pos]["out"] for pos in range(cpg)]
        outs.append(np.concatenate(cols, axis=1))
    return np.stack(outs, axis=0).astype(np.float32)


_NC_CACHE = {}


def _get_program():
    if "nc" not in _NC_CACHE:
        _NC_CACHE["nc"] = build_program()
    return _NC_CACHE["nc"]


def kernel(x, w_attn, w_proj, freqs, delta):
    nc = _get_program()
    in_maps = make_in_maps(x, w_attn, w_proj, freqs, delta)
    res = run_bass_kernel_spmd(nc, in_maps, list(range(N_CORES)))
    return assemble_output(res.results)


# revision 16
# speedup vs baseline: 1.4198x; 1.0320x over previous
"""Causal self-attention with anchor-relative rope (ferope), 8-core TRN2 Bass kernel.

Full-scale problem: B=2, T=2048, C=2048, H=16, D=128, M=32.

Sharding (tensor-parallel heads + data-parallel batch):
  - 8 cores = 2 batch groups x 4 cores. Core (b, g) handles batch b, heads 4g..4g+3.
  - qkv projection: each core computes q/k/v only for its heads (w_attn column shard),
    from x[b] transposed (host prep) so the contraction dim c sits on partitions.
  - attention computed with scores transposed: s_T[ki,qi], so both attention
    matmuls contract along partitions with no on-device transposes.
  - y_T head slices ([128, T] per head, c-major) are AllGathered within each
    4-core batch group -> y_all [C, T].
  - output projection is column-sharded: each core computes out[b][:, g*512:(g+1)*512].

All matmuls run as float32r (1 cycle/row at N>=512) except the qkv projection,
whose inputs (xT and w_attn shards) are cast to bf16 on device to fit SBUF.
"""

import math

import numpy as np

import concourse.bass as bass
import concourse.mybir as mybir
import concourse.tile as tile
from concourse import bacc
from concourse.bass_utils import run_bass_kernel_spmd

F32 = mybir.dt.float32
F32R = mybir.dt.float32r
BF16 = mybir.dt.bfloat16

# full-scale dims (hardcoded per harness contract)
B, T, C, H, DH, M = 2, 2048, 2048, 16, 128, 32
N_CORES = 8
GROUPS = 2                     # batch groups
CPG = N_CORES // GROUPS        # cores per group = 4
HPC = H // CPG                 # heads per core = 4
C_LOC = HPC * DH               # 512: per-core head channels
PANEL = 512                    # qi panel width (one psum bank)
KB = 128                       # ki block (partition dim)


def r(ap):
    """View a float32 AP as float32r for full-rate matmul."""
    return ap.bitcast(F32R)


def build_program(T=T, C=C, HPC=HPC, DH=DH, M=M, n_cores=N_CORES, groups=GROUPS):
    """Build the SPMD Bass program (same program on all cores; data differs)."""
    cpg = n_cores // groups
    c_loc = HPC * DH
    n_cb = C // KB            # contraction blocks for qkv/proj
    n_panels = T // PANEL
    n_tb = T // KB
    kb_per_panel = PANEL // KB  # 4
    inv_sqrt_d = 1.0 / math.sqrt(DH)

    nc = bacc.Bacc("TRN2", target_bir_lowering=False, debug=False,
                   num_devices=n_cores)

    xT_d = nc.dram_tensor("xT", [C, T], F32, kind="ExternalInput").ap()
    wqk_d = nc.dram_tensor("wqk", [C, 2 * c_loc], F32, kind="ExternalInput").ap()
    wv_d = nc.dram_tensor("wv", [C, c_loc], F32, kind="ExternalInput").ap()
    wo_d = nc.dram_tensor("wo", [C, c_loc], F32R, kind="ExternalInput").ap()
    freqs_d = nc.dram_tensor("freqs", [M], F32, kind="ExternalInput").ap()
    delta_d = nc.dram_tensor("delta", [T], F32, kind="ExternalInput").ap()
    out_d = nc.dram_tensor("out", [T, c_loc], F32, kind="ExternalOutput").ap()

    replica_groups = [list(range(g * cpg, (g + 1) * cpg)) for g in range(groups)]

    with tile.TileContext(nc) as tc:
        with (
            tc.tile_pool(name="dram", bufs=1, space="DRAM") as dram,
            tc.tile_pool(name="const", bufs=1) as const,
            tc.tile_pool(name="qkv", bufs=1) as qkv,
            tc.tile_pool(name="work", bufs=1) as work,
        ):
            y_part = dram.tile([c_loc, T], F32R)
            y_all = dram.tile([cpg * c_loc, T], F32R)

            # ---- constants: trig tables, causal masks, ones ----
            ones128 = const.tile([KB, KB], BF16)
            nc.vector.memset(ones128[:], 1.0)

            sinN = const.tile([2 * M, T], F32)
            cos64 = const.tile([2 * M, T], F32)
            masks = [const.tile([KB, PANEL], BF16, name=f"maskf{p}")
                     for p in range(kb_per_panel)]
            with tc.tile_pool(name="setup", bufs=1) as setup:
                # fr64 = [-freqs; freqs] as per-partition scalars
                fr64 = setup.tile([2 * M, 1], F32)
                nc.sync.dma_start(out=fr64[0:M, :],
                                  in_=freqs_d.rearrange("m -> m ()"))
                nc.sync.dma_start(out=fr64[M:2 * M, :],
                                  in_=freqs_d.rearrange("m -> m ()"))
                nc.vector.tensor_scalar_mul(fr64[0:M, :], fr64[0:M, :], -1.0)

                # delta broadcast across 2M partitions
                delta_row = setup.tile([1, T], F32)
                nc.sync.dma_start(out=delta_row[:],
                                  in_=delta_d.rearrange("t -> () t"))
                delta_rep = setup.tile([2 * M, T], F32)
                nc.gpsimd.partition_broadcast(delta_rep[:], delta_row[:],
                                              channels=2 * M)

                # ang = delta * (+-freqs); sinN = [-sin; sin], cos = [cos; cos]
                ang = setup.tile([2 * M, T], F32)
                nc.vector.tensor_scalar_mul(ang[:], delta_rep[:], fr64[:])
                nc.scalar.activation(sinN[:], ang[:],
                                     mybir.ActivationFunctionType.Sin)
                pi2 = setup.tile([2 * M, 1], F32)
                nc.vector.memset(pi2[:], math.pi / 2)
                nc.scalar.activation(cos64[:], ang[:],
                                     mybir.ActivationFunctionType.Sin,
                                     bias=pi2[:])

                # causal masks for diagonal tiles: mask_p = (qi >= ki + 128*p)
                for p in range(kb_per_panel):
                    mi = setup.tile([KB, PANEL], F32, tag="maski", bufs=2,
                                    name=f"maski{p}")
                    nc.gpsimd.iota(mi[:], pattern=[[1, PANEL]], base=-KB * p,
                                   channel_multiplier=-1,
                                   allow_small_or_imprecise_dtypes=True)
                    nc.vector.tensor_scalar(masks[p][:], mi[:], 0.0, None,
                                            mybir.AluOpType.is_ge)

            # ---- qkv projection for all local heads, single pass over xT ----
            # q/k stored per head as [d, t] bf16; v natural [t, d] bf16.
            q_sb = [qkv.tile([DH, T], BF16, name=f"q{h}") for h in range(HPC)]
            k_sb = [qkv.tile([DH, T], BF16, name=f"k{h}") for h in range(HPC)]
            v_all = qkv.tile([KB, n_tb, c_loc], BF16)
            with tc.tile_pool(name="wload", bufs=1) as wload:
                CH = 128  # staging chunk width (columns)

                def load_bf16(dst3, src_t, width, name):
                    """Chunked DRAM->SBUF load of [C, width] slab (kb-tiled
                    3D view src_t [p, kb, width]) into bf16 tile dst3."""
                    for ci in range(width // CH):
                        st = wload.tile([KB, n_cb, CH], F32, tag="stage3",
                                        bufs=2, name=f"st_{name}{ci}")
                        nc.sync.dma_start(
                            out=st[:],
                            in_=src_t[:, :, ci * CH:(ci + 1) * CH])
                        nc.vector.tensor_copy(
                            dst3[:, :, ci * CH:(ci + 1) * CH], st[:])

                wv_t = wv_d.rearrange("(kb p) c -> p kb c", p=KB)
                wvb3 = wload.tile([KB, n_cb, c_loc], BF16, tag="wvbf")
                load_bf16(wvb3, wv_t, c_loc, "wv")
                wvb = [wvb3[:, i, :] for i in range(n_cb)]
                wqk_t = wqk_d.rearrange("(kb p) c -> p kb c", p=KB)
                wqkb3 = wload.tile([KB, n_cb, 2 * c_loc], BF16, tag="wqkbf")
                load_bf16(wqkb3, wqk_t, 2 * c_loc, "wqk")
                wqkb = [wqkb3[:, i, :] for i in range(n_cb)]
                xT_t = xT_d.rearrange("(kb p) t -> p kb t", p=KB)
                with tc.tile_pool(name="psq", bufs=1, space="PSUM") as psq:
                    for tp in range(n_panels):
                        tps = tp * PANEL
                        xb3 = wload.tile([KB, n_cb, PANEL], BF16, tag="xbf",
                                         bufs=2, name=f"xb{tp}")
                        for ci in range(PANEL // CH):
                            st = wload.tile([KB, n_cb, CH], F32, tag="stage3",
                                            bufs=2, name=f"st_x{tp}_{ci}")
                            nc.sync.dma_start(
                                out=st[:],
                                in_=xT_t[:, :, tps + ci * CH:
                                         tps + (ci + 1) * CH])
                            nc.vector.tensor_copy(
                                xb3[:, :, ci * CH:(ci + 1) * CH], st[:])
                        xbf = [xb3[:, kb, :] for kb in range(n_cb)]
                        # v blocks for the 128-rows inside this panel
                        for tbl in range(kb_per_panel):
                            tb = tp * kb_per_panel + tbl
                            pv = psq.tile([KB, c_loc], F32, tag="v", bufs=3)
                            for kb in range(n_cb):
                                nc.tensor.matmul(
                                    pv[:],
                                    xbf[kb][:, tbl * KB:(tbl + 1) * KB],
                                    wvb[kb],
                                    start=(kb == 0), stop=(kb == n_cb - 1))
                            nc.scalar.copy(v_all[:, tb, :], pv[:])
                        # q/k column blocks: cb<HPC -> q head cb; else k head
                        for cb in range(2 * HPC):
                            pqk = psq.tile([DH, PANEL], F32, tag="qk", bufs=3)
                            for kb in range(n_cb):
                                nc.tensor.matmul(
                                    pqk[:],
                                    wqkb[kb][:, cb * DH:(cb + 1) * DH],
                                    xbf[kb],
                                    start=(kb == 0), stop=(kb == n_cb - 1))
                            dst = q_sb[cb] if cb < HPC else k_sb[cb - HPC]
                            nc.scalar.copy(dst[:, tps:tps + PANEL], pqk[:])

            # ---- rope on rows 0:2M of each q/k head ----
            for u in [t for pair in zip(q_sb, k_sb) for t in pair]:
                sw = work.tile([2 * M, T], BF16, tag="ropesw", bufs=2)
                nc.sync.dma_start(out=sw[0:M, :], in_=u[M:2 * M, :])
                nc.sync.dma_start(out=sw[M:2 * M, :], in_=u[0:M, :])
                nc.vector.tensor_mul(sw[:], sw[:], sinN[:])
                nc.vector.tensor_mul(u[0:2 * M, :], u[0:2 * M, :], cos64[:])
                nc.vector.tensor_add(u[0:2 * M, :], u[0:2 * M, :], sw[:])

            # ---- causal attention per head + per-head AllGather ----
            # y_all rows are head-major: (head, group, d) so each per-head
            # gather writes one contiguous [cpg*DH, T] block
            with tc.tile_pool(name="proj", bufs=1) as proj:
                # prefetch proj weights during attention
                wob = []
                for i in range(n_cb):
                    wo_sb = proj.tile([KB, c_loc], F32R, tag="wo", bufs=n_cb,
                                      name=f"wo{i}")
                    nc.sync.dma_start(out=wo_sb[:],
                                      in_=wo_d[i * KB:(i + 1) * KB, :])
                    wob.append(wo_sb)
                psa_cm = tc.tile_pool(name="psa", bufs=1, space="PSUM")
                psa = psa_cm.__enter__()
                for h in range(HPC):
                    qh, kh = q_sb[h], k_sb[h]
                    for J in range(n_panels):
                        nkb = (J + 1) * kb_per_panel
                        py = psa.tile([DH, PANEL], F32, tag="y", bufs=3)
                        pr = psa.tile([KB, PANEL], F32, tag="r", bufs=2)
                        qs = J * PANEL
                        for b in range(nkb):
                            ps = psa.tile([KB, PANEL], F32, tag="s", bufs=3)
                            nc.tensor.matmul(
                                ps[:],
                                kh[:, b * KB:(b + 1) * KB],
                                qh[:, qs:qs + PANEL],
                                start=True, stop=True)
                            et = work.tile([KB, PANEL], BF16, tag="exp",
                                           bufs=4)
                            nc.scalar.activation(
                                et[:], ps[:],
                                mybir.ActivationFunctionType.Exp,
                                scale=inv_sqrt_d)
                            p = b - kb_per_panel * J
                            if p >= 0:
                                nc.vector.tensor_mul(et[:], et[:], masks[p][:])
                            nc.tensor.matmul(
                                py[:],
                                v_all[:, b, h * DH:(h + 1) * DH],
                                et[:],
                                start=(b == 0), stop=(b == nkb - 1))
                            # rowsum, pre-replicated across partitions by
                            # using an all-ones [128,128] stationary operand
                            nc.tensor.matmul(
                                pr[:], ones128[:], et[:],
                                start=(b == 0), stop=(b == nkb - 1))
                        # normalize: y * (1/rowsum)
                        rep = work.tile([KB, PANEL], F32, tag="rep_sb", bufs=2)
                        nc.scalar.copy(rep[:], pr[:])
                        rinv = work.tile([KB, PANEL], F32, tag="rinv", bufs=2)
                        nc.vector.reciprocal_approx_fast(rinv[:], rep[:])
                        ysb = work.tile([DH, PANEL], F32, tag="ysb", bufs=3)
                        nc.vector.tensor_mul(ysb[:], py[:], rinv[:])
                        nc.sync.dma_start(
                            out=y_part[h * DH:(h + 1) * DH, qs:qs + PANEL],
                            in_=r(ysb[:]))
                    # gather this head's y slice across the batch group
                    nc.gpsimd.collective_compute(
                        "AllGather",
                        mybir.AluOpType.bypass,
                        replica_groups=replica_groups,
                        ins=[y_part[h * DH:(h + 1) * DH, :]],
                        outs=[y_all[h * cpg * DH:(h + 1) * cpg * DH, :]],
                    )

                psa_cm.__exit__(None, None, None)
                # ---- out[:, local cols] = y_all.T @ wo ----
                # Accumulate over heads as each head's AllGather lands:
                # two half-T passes so the 8 in-flight [t,512] accumulators
                # fit in the 8 psum banks. y_all row (hh, g, p), cb = g*HPC+hh.
                y_all_tiled = y_all[:].rearrange(
                    "(hh g p) t -> p hh g t", hh=HPC, g=cpg)
                half = n_tb // 2
                with tc.tile_pool(name="pso", bufs=1, space="PSUM") as pso:
                    for ph in range(2):
                        pos = [pso.tile([KB, c_loc], F32, tag=f"o{i}", bufs=1,
                                        name=f"po{ph}_{i}")
                               for i in range(half)]
                        for hh in range(HPC):
                            for i in range(half):
                                tb = ph * half + i
                                yt = proj.tile([KB, cpg, KB], F32R, tag="yt",
                                               bufs=4)
                                nc.sync.dma_start(
                                    out=yt[:],
                                    in_=y_all_tiled[:, hh, :,
                                                    tb * KB:(tb + 1) * KB])
                                for g in range(cpg):
                                    nc.tensor.matmul(
                                        po_ := pos[i][:], yt[:, g, :],
                                        wob[g * HPC + hh][:],
                                        start=(hh == 0 and g == 0),
                                        stop=(hh == HPC - 1 and g == cpg - 1))
                        for i in range(half):
                            tb = ph * half + i
                            osb = proj.tile([KB, c_loc], F32, tag="osb",
                                            bufs=3)
                            nc.scalar.copy(osb[:], pos[i][:])
                            nc.sync.dma_start(
                                out=out_d[tb * KB:(tb + 1) * KB, :],
                                in_=osb[:])

    nc.compile()
    return nc


def make_in_maps(x, w_attn, w_proj, freqs, delta, n_cores=N_CORES,
                 groups=GROUPS, dh=DH):
    """Host-side sharding: slice/transpose full inputs into per-core maps."""
    x = np.asarray(x, dtype=np.float32)
    w_attn = np.asarray(w_attn, dtype=np.float32)
    w_proj = np.asarray(w_proj, dtype=np.float32)
    freqs = np.asarray(freqs, dtype=np.float32)
    delta = np.asarray(delta, dtype=np.float32)
    b_, t_, c_ = x.shape
    cpg = n_cores // groups
    h_ = w_attn.shape[1] // (3 * dh)
    hpc = h_ // cpg
    c_loc = hpc * dh
    in_maps = []
    for core in range(n_cores):
        g, pos = divmod(core, cpg)
        heads = range(pos * hpc, (pos + 1) * hpc)
        xT = np.ascontiguousarray(x[g].T)
        wqk = np.concatenate(
            [w_attn[:, h * dh:(h + 1) * dh] for h in heads]
            + [w_attn[:, c_ + h * dh:c_ + (h + 1) * dh] for h in heads], axis=1)
        wv = np.concatenate(
            [w_attn[:, 2 * c_ + h * dh:2 * c_ + (h + 1) * dh] for h in heads],
            axis=1)
        wo = np.ascontiguousarray(w_proj[:, pos * c_loc:(pos + 1) * c_loc])
        in_maps.append({
            "xT": xT,
            "wqk": np.ascontiguousarray(wqk),
            "wv": np.ascontiguousarray(wv),
            "wo": wo,
            "freqs": freqs,
            "delta": delta,
        })
    return in_maps


def assemble_output(results, n_cores=N_CORES, groups=GROUPS):
    cpg = n_cores // groups
    outs = []
    for g in range(groups):
        cols = [results[g * cpg + pos]["out"] for pos in range(cpg)]
        outs.append(np.concatenate(cols, axis=1))
    return np.stack(outs, axis=0).astype(np.float32)


_NC_CACHE = {}


def _get_program():
    if "nc" not in _NC_CACHE:
        _NC_CACHE["nc"] = build_program()
    return _NC_CACHE["nc"]


def kernel(x, w_attn, w_proj, freqs, delta):
    nc = _get_program()
    in_maps = make_in_maps(x, w_attn, w_proj, freqs, delta)
    res = run_bass_kernel_spmd(nc, in_maps, list(range(N_CORES)))
    return assemble_output(res.results)


# revision 18
# speedup vs baseline: 1.7051x; 1.2010x over previous
"""Causal self-attention with anchor-relative rope (ferope), 8-core TRN2 Bass kernel.

Full-scale problem: B=2, T=2048, C=2048, H=16, D=128, M=32.

Sharding (tensor-parallel heads + data-parallel batch):
  - 8 cores = 2 batch groups x 4 cores. Core (b, g) handles batch b, heads 4g..4g+3.
  - qkv projection: each core computes q/k/v only for its heads (w_attn column shard),
    from x[b] transposed (host prep) so the contraction dim c sits on partitions.
  - attention computed with scores transposed: s_T[ki,qi], so both attention
    matmuls contract along partitions with no on-device transposes.
  - y_T head slices ([128, T] per head, c-major) are AllGathered within each
    4-core batch group -> y_all [C, T].
  - output projection is column-sharded: each core computes out[b][:, g*512:(g+1)*512].

All matmuls run as float32r (1 cycle/row at N>=512) except the qkv projection,
whose inputs (xT and w_attn shards) are cast to bf16 on device to fit SBUF.
"""

import math

import numpy as np

import concourse.bass as bass
import concourse.mybir as mybir
import concourse.tile as tile
from concourse import bacc
from concourse.bass_utils import run_bass_kernel_spmd

F32 = mybir.dt.float32
F32R = mybir.dt.float32r
BF16 = mybir.dt.bfloat16

# full-scale dims (hardcoded per harness contract)
B, T, C, H, DH, M = 2, 2048, 2048, 16, 128, 32
N_CORES = 8
GROUPS = 2                     # batch groups
CPG = N_CORES // GROUPS        # cores per group = 4
HPC = H // CPG                 # heads per core = 4
C_LOC = HPC * DH               # 512: per-core head channels
PANEL = 512                    # qi panel width (one psum bank)
KB = 128                       # ki block (partition dim)


def r(ap):
    """View a float32 AP as float32r for full-rate matmul."""
    return ap.bitcast(F32R)


def build_program(T=T, C=C, HPC=HPC, DH=DH, M=M, n_cores=N_CORES, groups=GROUPS):
    """Build the SPMD Bass program (same program on all cores; data differs)."""
    cpg = n_cores // groups
    c_loc = HPC * DH
    n_cb = C // KB            # contraction blocks for qkv/proj
    n_panels = T // PANEL
    n_tb = T // KB
    kb_per_panel = PANEL // KB  # 4
    inv_sqrt_d = 1.0 / math.sqrt(DH)

    nc = bacc.Bacc("TRN2", target_bir_lowering=False, debug=False,
                   num_devices=n_cores)

    xT_d = nc.dram_tensor("xT", [C, T], F32, kind="ExternalInput").ap()
    wqk_d = nc.dram_tensor("wqk", [C, 2 * c_loc], F32, kind="ExternalInput").ap()
    wv_d = nc.dram_tensor("wv", [C, c_loc], F32, kind="ExternalInput").ap()
    wo_d = nc.dram_tensor("wo", [C, c_loc], F32, kind="ExternalInput").ap()
    freqs_d = nc.dram_tensor("freqs", [M], F32, kind="ExternalInput").ap()
    delta_d = nc.dram_tensor("delta", [T], F32, kind="ExternalInput").ap()
    out_d = nc.dram_tensor("out", [T, c_loc], F32, kind="ExternalOutput").ap()

    replica_groups = [list(range(g * cpg, (g + 1) * cpg)) for g in range(groups)]

    with tile.TileContext(nc) as tc:
        with (
            tc.tile_pool(name="dram", bufs=1, space="DRAM") as dram,
            tc.tile_pool(name="const", bufs=1) as const,
            tc.tile_pool(name="qkv", bufs=1) as qkv,
            tc.tile_pool(name="work", bufs=1) as work,
        ):
            y_part = dram.tile([c_loc, T], BF16)
            y_all = dram.tile([cpg * c_loc, T], BF16)

            # ---- constants: trig tables, causal masks, ones ----
            ones128 = const.tile([KB, KB], BF16)
            nc.vector.memset(ones128[:], 1.0)

            sinN = const.tile([2 * M, T], F32)
            cos64 = const.tile([2 * M, T], F32)
            masks = [const.tile([KB, PANEL], BF16, name=f"maskf{p}")
                     for p in range(kb_per_panel)]
            with tc.tile_pool(name="setup", bufs=1) as setup:
                # fr64 = [-freqs; freqs] as per-partition scalars
                fr64 = setup.tile([2 * M, 1], F32)
                nc.sync.dma_start(out=fr64[0:M, :],
                                  in_=freqs_d.rearrange("m -> m ()"))
                nc.sync.dma_start(out=fr64[M:2 * M, :],
                                  in_=freqs_d.rearrange("m -> m ()"))
                nc.vector.tensor_scalar_mul(fr64[0:M, :], fr64[0:M, :], -1.0)

                # delta broadcast across 2M partitions
                delta_row = setup.tile([1, T], F32)
                nc.sync.dma_start(out=delta_row[:],
                                  in_=delta_d.rearrange("t -> () t"))
                delta_rep = setup.tile([2 * M, T], F32)
                nc.gpsimd.partition_broadcast(delta_rep[:], delta_row[:],
                                              channels=2 * M)

                # ang = delta * (+-freqs); sinN = [-sin; sin], cos = [cos; cos]
                ang = setup.tile([2 * M, T], F32)
                nc.vector.tensor_scalar_mul(ang[:], delta_rep[:], fr64[:])
                nc.scalar.activation(sinN[:], ang[:],
                                     mybir.ActivationFunctionType.Sin)
                pi2 = setup.tile([2 * M, 1], F32)
                nc.vector.memset(pi2[:], math.pi / 2)
                nc.scalar.activation(cos64[:], ang[:],
                                     mybir.ActivationFunctionType.Sin,
                                     bias=pi2[:])

                # causal masks for diagonal tiles: mask_p = (qi >= ki + 128*p)
                for p in range(kb_per_panel):
                    mi = setup.tile([KB, PANEL], F32, tag="maski", bufs=2,
                                    name=f"maski{p}")
                    nc.gpsimd.iota(mi[:], pattern=[[1, PANEL]], base=-KB * p,
                                   channel_multiplier=-1,
                                   allow_small_or_imprecise_dtypes=True)
                    nc.vector.tensor_scalar(masks[p][:], mi[:], 0.0, None,
                                            mybir.AluOpType.is_ge)

            # ---- qkv projection for all local heads, single pass over xT ----
            # q/k stored per head as [d, t] bf16; v natural [t, d] bf16.
            q_sb = [qkv.tile([DH, T], BF16, name=f"q{h}") for h in range(HPC)]
            k_sb = [qkv.tile([DH, T], BF16, name=f"k{h}") for h in range(HPC)]
            v_all = qkv.tile([KB, n_tb, c_loc], BF16)
            with tc.tile_pool(name="wload", bufs=1) as wload:
                STAGE_ELEMS = 4 * 512  # f32 staging slot: 8KB/partition

                def load_bf16(dst3, src_t, width, name):
                    """Chunked DRAM->SBUF load of a [C, width] slab (kb-tiled
                    3D view src_t [p, kb, width]) into bf16 tile dst3,
                    chunking along kb so DMA rows stay >=2KB."""
                    kbc = min(max(STAGE_ELEMS // width, 1), n_cb)
                    for ci in range((n_cb + kbc - 1) // kbc):
                        lo = ci * kbc
                        hi = min(lo + kbc, n_cb)
                        st = wload.tile([KB, hi - lo, width], F32,
                                        tag="stage3", bufs=2,
                                        name=f"st_{name}{ci}")
                        nc.sync.dma_start(out=st[:], in_=src_t[:, lo:hi, :])
                        nc.vector.tensor_copy(dst3[:, lo:hi, :], st[:])

                wv_t = wv_d.rearrange("(kb p) c -> p kb c", p=KB)
                wvb3 = wload.tile([KB, n_cb, c_loc], BF16, tag="wvbf")
                load_bf16(wvb3, wv_t, c_loc, "wv")
                wvb = [wvb3[:, i, :] for i in range(n_cb)]
                wqk_t = wqk_d.rearrange("(kb p) c -> p kb c", p=KB)
                wqkb3 = wload.tile([KB, n_cb, 2 * c_loc], BF16, tag="wqkbf")
                load_bf16(wqkb3, wqk_t, 2 * c_loc, "wqk")
                wqkb = [wqkb3[:, i, :] for i in range(n_cb)]
                xT_t = xT_d.rearrange("(kb p) t -> p kb t", p=KB)
                with tc.tile_pool(name="psq", bufs=1, space="PSUM") as psq:
                    for tp in range(n_panels):
                        tps = tp * PANEL
                        xb3 = wload.tile([KB, n_cb, PANEL], BF16, tag="xbf",
                                         bufs=2, name=f"xb{tp}")
                        kbc = min(max(STAGE_ELEMS // PANEL, 1), n_cb)
                        for ci in range((n_cb + kbc - 1) // kbc):
                            lo = ci * kbc
                            hi = min(lo + kbc, n_cb)
                            st = wload.tile([KB, hi - lo, PANEL], F32,
                                            tag="stage3", bufs=2,
                                            name=f"st_x{tp}_{ci}")
                            nc.sync.dma_start(
                                out=st[:],
                                in_=xT_t[:, lo:hi, tps:tps + PANEL])
                            nc.vector.tensor_copy(
                                xb3[:, lo:hi, :], st[:])
                        xbf = [xb3[:, kb, :] for kb in range(n_cb)]
                        # v blocks for the 128-rows inside this panel
                        for tbl in range(kb_per_panel):
                            tb = tp * kb_per_panel + tbl
                            pv = psq.tile([KB, c_loc], F32, tag="v", bufs=3)
                            for kb in range(n_cb):
                                nc.tensor.matmul(
                                    pv[:],
                                    xbf[kb][:, tbl * KB:(tbl + 1) * KB],
                                    wvb[kb],
                                    start=(kb == 0), stop=(kb == n_cb - 1))
                            nc.scalar.copy(v_all[:, tb, :], pv[:])
                        # q/k column blocks: cb<HPC -> q head cb; else k head
                        for cb in range(2 * HPC):
                            pqk = psq.tile([DH, PANEL], F32, tag="qk", bufs=3)
                            for kb in range(n_cb):
                                nc.tensor.matmul(
                                    pqk[:],
                                    wqkb[kb][:, cb * DH:(cb + 1) * DH],
                                    xbf[kb],
                                    start=(kb == 0), stop=(kb == n_cb - 1))
                            dst = q_sb[cb] if cb < HPC else k_sb[cb - HPC]
                            nc.scalar.copy(dst[:, tps:tps + PANEL], pqk[:])

            # ---- rope on rows 0:2M of each q/k head ----
            for u in [t for pair in zip(q_sb, k_sb) for t in pair]:
                sw = work.tile([2 * M, T], BF16, tag="ropesw", bufs=2)
                nc.sync.dma_start(out=sw[0:M, :], in_=u[M:2 * M, :])
                nc.sync.dma_start(out=sw[M:2 * M, :], in_=u[0:M, :])
                nc.vector.tensor_mul(sw[:], sw[:], sinN[:])
                nc.vector.tensor_mul(u[0:2 * M, :], u[0:2 * M, :], cos64[:])
                nc.vector.tensor_add(u[0:2 * M, :], u[0:2 * M, :], sw[:])

            # ---- causal attention per head + per-head AllGather ----
            # y_all rows are head-major: (head, group, d) so each per-head
            # gather writes one contiguous [cpg*DH, T] block
            with tc.tile_pool(name="proj", bufs=1) as proj:
                # prefetch proj weights during attention
                wob = []
                for i in range(n_cb):
                    wo_st = proj.tile([KB, c_loc], F32, tag="wost", bufs=2,
                                      name=f"wost{i}")
                    nc.sync.dma_start(out=wo_st[:],
                                      in_=wo_d[i * KB:(i + 1) * KB, :])
                    wo_sb = proj.tile([KB, c_loc], BF16, tag="wo", bufs=n_cb,
                                      name=f"wo{i}")
                    nc.vector.tensor_copy(wo_sb[:], wo_st[:])
                    wob.append(wo_sb)
                psa_cm = tc.tile_pool(name="psa", bufs=1, space="PSUM")
                psa = psa_cm.__enter__()
                for h in range(HPC):
                    qh, kh = q_sb[h], k_sb[h]
                    for J in range(n_panels):
                        nkb = (J + 1) * kb_per_panel
                        py = psa.tile([DH, PANEL], F32, tag="y", bufs=3)
                        pr = psa.tile([KB, PANEL], F32, tag="r", bufs=2)
                        qs = J * PANEL
                        for b in range(nkb):
                            ps = psa.tile([KB, PANEL], F32, tag="s", bufs=3)
                            nc.tensor.matmul(
                                ps[:],
                                kh[:, b * KB:(b + 1) * KB],
                                qh[:, qs:qs + PANEL],
                                start=True, stop=True)
                            et = work.tile([KB, PANEL], BF16, tag="exp",
                                           bufs=4)
                            nc.scalar.activation(
                                et[:], ps[:],
                                mybir.ActivationFunctionType.Exp,
                                scale=inv_sqrt_d)
                            p = b - kb_per_panel * J
                            if p >= 0:
                                nc.vector.tensor_mul(et[:], et[:], masks[p][:])
                            nc.tensor.matmul(
                                py[:],
                                v_all[:, b, h * DH:(h + 1) * DH],
                                et[:],
                                start=(b == 0), stop=(b == nkb - 1))
                            # rowsum, pre-replicated across partitions by
                            # using an all-ones [128,128] stationary operand
                            nc.tensor.matmul(
                                pr[:], ones128[:], et[:],
                                start=(b == 0), stop=(b == nkb - 1))
                        # normalize: y * (1/rowsum)
                        rep = work.tile([KB, PANEL], F32, tag="rep_sb", bufs=2)
                        nc.scalar.copy(rep[:], pr[:])
                        rinv = work.tile([KB, PANEL], F32, tag="rinv", bufs=2)
                        nc.vector.reciprocal_approx_fast(rinv[:], rep[:])
                        ysb = work.tile([DH, PANEL], BF16, tag="ysb", bufs=3)
                        nc.vector.tensor_mul(ysb[:], py[:], rinv[:])
                        nc.sync.dma_start(
                            out=y_part[h * DH:(h + 1) * DH, qs:qs + PANEL],
                            in_=ysb[:])
                    # gather this head's y slice across the batch group
                    nc.gpsimd.collective_compute(
                        "AllGather",
                        mybir.AluOpType.bypass,
                        replica_groups=replica_groups,
                        ins=[y_part[h * DH:(h + 1) * DH, :]],
                        outs=[y_all[h * cpg * DH:(h + 1) * cpg * DH, :]],
                    )

                psa_cm.__exit__(None, None, None)
                # ---- out[:, local cols] = y_all.T @ wo ----
                # Accumulate over heads as each head's AllGather lands:
                # two half-T passes so the 8 in-flight [t,512] accumulators
                # fit in the 8 psum banks. y_all row (hh, g, p), cb = g*HPC+hh.
                y_all_tiled = y_all[:].rearrange(
                    "(hh g p) t -> p hh g t", hh=HPC, g=cpg)
                half = n_tb // 2
                with tc.tile_pool(name="pso", bufs=1, space="PSUM") as pso:
                    for ph in range(2):
                        pos = [pso.tile([KB, c_loc], F32, tag=f"o{i}", bufs=1,
                                        name=f"po{ph}_{i}")
                               for i in range(half)]
                        for hh in range(HPC):
                            for i in range(half):
                                tb = ph * half + i
                                yt = proj.tile([KB, cpg, KB], BF16, tag="yt",
                                               bufs=4)
                                nc.sync.dma_start(
                                    out=yt[:],
                                    in_=y_all_tiled[:, hh, :,
                                                    tb * KB:(tb + 1) * KB])
                                for g in range(cpg):
                                    nc.tensor.matmul(
                                        po_ := pos[i][:], yt[:, g, :],
                                        wob[g * HPC + hh][:],
                                        start=(hh == 0 and g == 0),
                                        stop=(hh == HPC - 1 and g == cpg - 1))
                        for i in range(half):
                            tb = ph * half + i
                            osb = proj.tile([KB, c_loc], F32, tag="osb",
                                            bufs=3)
                            nc.scalar.copy(osb[:], pos[i][:])
                            nc.sync.dma_start(
                                out=out_d[tb * KB:(tb + 1) * KB, :],
                                in_=osb[:])

    nc.compile()
    return nc


def make_in_maps(x, w_attn, w_proj, freqs, delta, n_cores=N_CORES,
                 groups=GROUPS, dh=DH):
    """Host-side sharding: slice/transpose full inputs into per-core maps."""
    x = np.asarray(x, dtype=np.float32)
    w_attn = np.asarray(w_attn, dtype=np.float32)
    w_proj = np.asarray(w_proj, dtype=np.float32)
    freqs = np.asarray(freqs, dtype=np.float32)
    delta = np.asarray(delta, dtype=np.float32)
    b_, t_, c_ = x.shape
    cpg = n_cores // groups
    h_ = w_attn.shape[1] // (3 * dh)
    hpc = h_ // cpg
    c_loc = hpc * dh
    in_maps = []
    for core in range(n_cores):
        g, pos = divmod(core, cpg)
        heads = range(pos * hpc, (pos + 1) * hpc)
        xT = np.ascontiguousarray(x[g].T)
        wqk = np.concatenate(
            [w_attn[:, h * dh:(h + 1) * dh] for h in heads]
            + [w_attn[:, c_ + h * dh:c_ + (h + 1) * dh] for h in heads], axis=1)
        wv = np.concatenate(
            [w_attn[:, 2 * c_ + h * dh:2 * c_ + (h + 1) * dh] for h in heads],
            axis=1)
        wo = np.ascontiguousarray(w_proj[:, pos * c_loc:(pos + 1) * c_loc])
        in_maps.append({
            "xT": xT,
            "wqk": np.ascontiguousarray(wqk),
            "wv": np.ascontiguousarray(wv),
            "wo": wo,
            "freqs": freqs,
            "delta": delta,
        })
    return in_maps


def assemble_output(results, n_cores=N_CORES, groups=GROUPS):
    cpg = n_cores // groups
    outs = []
    for g in range(groups):
        cols = [results[g * cpg + pos]["out"] for pos in range(cpg)]
        outs.append(np.concatenate(cols, axis=1))
    return np.stack(outs, axis=0).astype(np.float32)


_NC_CACHE = {}


def _get_program():
    if "nc" not in _NC_CACHE:
        _NC_CACHE["nc"] = build_program()
    return _NC_CACHE["nc"]


def kernel(x, w_attn, w_proj, freqs, delta):
    nc = _get_program()
    in_maps = make_in_maps(x, w_attn, w_proj, freqs, delta)
    res = run_bass_kernel_spmd(nc, in_maps, list(range(N_CORES)))
    return assemble_output(res.results)
